# revision 27
# baseline (speedup 1.0000x reference)
"""Trainium2 Bass kernel for nn_Decoder_LSTM: 12-step LSTM over (16, 10000, 64).

Sharding: rows = B*N = 160000 flattened, 20000 rows per core (data-parallel);
gate + edge weights replicated on all 8 cores.

Per-core layout: two 10000-row halves (A, B) packed into 128 partitions,
feature-major:
  XA/XB (128, 10000) bf16 : partitions 0:64 = x^T, 64:128 = h^T
  H     (128, 10000) bf16 : partitions 0:64 = hA,  64:128 = hB
  C     (128, 10000) bf16 : cell state, same packing as H

All four gate activations are sigmoids (tanh(g) = 2*sigmoid(2g)-1 with the
2x folded into the g-gate weights/bias), so each gate's PSUM tile (4 banks,
2048 cols) is drained by ONE ScalarE sigmoid with the per-gate bias riding
the per-partition bias operand.  Gates are time-multiplexed through a
2-buffer PSUM pool; per gate-bank two M=64 matmuls write the A-half rows to
partitions 0:64 and the B-half rows to 64:128 (no zero-padded weights, no
accumulation groups).  y = sigmoid(H @ blockdiag(We, We)) is one matmul per
bank from the shared H tile.  Output leaves the core as bf16 (T, 128, 10000)
and is unpacked/upcast on the host.
"""
import numpy as np

T, B, N, F = 12, 16, 10000, 64
R_TOTAL = B * N
N_CORES = 8
R = R_TOTAL // N_CORES   # 20000 rows per core
RH = R // 2              # 10000 per half
SC = 2048                # pair-columns per super-chunk
CHUNKS = [(i * 2000, 2000) for i in range(RH // 2000)]

_NC = None
LAST_EXEC_NS = None


def _bank_regions(cw):
    regs = []
    b0 = 0
    while b0 < cw:
        regs.append((b0, min(512, cw - b0)))
        b0 += 512
    return regs


def _build():
    from contextlib import ExitStack
    from concourse import bacc, mybir
    import concourse.tile as tile

    f32 = mybir.dt.float32
    bf16 = mybir.dt.bfloat16
    AF = mybir.ActivationFunctionType
    ALU = mybir.AluOpType

    nc = bacc.Bacc(trn_type="TRN2")
    xa_in = nc.dram_tensor("xa", [F, RH], bf16, kind="ExternalInput")
    xb_in = nc.dram_tensor("xb", [F, RH], bf16, kind="ExternalInput")
    gw_in = nc.dram_tensor("gw", [128, 256], bf16, kind="ExternalInput")
    we_in = nc.dram_tensor("we", [128, 128], bf16, kind="ExternalInput")
    bias_in = nc.dram_tensor("bias", [128, 4], f32, kind="ExternalInput")
    out = nc.dram_tensor("out", [T, 128, RH], bf16, kind="ExternalOutput")

    # gate order in the weight packing: g first so DVE can start on TG early
    # gw columns: 0:64 = g(x2), 64:128 = i, 128:192 = f, 192:256 = o
    QG, QI, QF, QO = 0, 1, 2, 3

    with tile.TileContext(nc) as tc, ExitStack() as ctx:
        fixed = ctx.enter_context(tc.tile_pool(name="fixed", bufs=1))
        state = ctx.enter_context(tc.tile_pool(name="state", bufs=1))
        work = ctx.enter_context(tc.tile_pool(name="work", bufs=2))
        psum = ctx.enter_context(tc.tile_pool(name="psum", bufs=2, space="PSUM"))

        XA = state.tile([128, RH], bf16, name="XA")
        XB = state.tile([128, RH], bf16, name="XB")
        H = state.tile([128, RH], bf16, name="H")
        C = state.tile([128, RH], bf16, name="C")
        GW = fixed.tile([128, 256], bf16)
        WE = fixed.tile([128, 128], bf16)
        BIAS = fixed.tile([128, 4], f32)

        # first-needed data first: x chunk 0, weights, then the rest
        c0_, cw_ = CHUNKS[0]
        nc.sync.dma_start(XA[0:64, c0_:c0_ + cw_], xa_in[:, c0_:c0_ + cw_])
        nc.sync.dma_start(XB[0:64, c0_:c0_ + cw_], xb_in[:, c0_:c0_ + cw_])
        nc.sync.dma_start(GW[:], gw_in[:])
        nc.sync.dma_start(BIAS[:], bias_in[:])
        nc.sync.dma_start(WE[:], we_in[:])
        for c0, cw in CHUNKS[1:]:
            nc.sync.dma_start(XA[0:64, c0:c0 + cw], xa_in[:, c0:c0 + cw])
            nc.sync.dma_start(XB[0:64, c0:c0 + cw], xb_in[:, c0:c0 + cw])

        # PE p-state warmup: ~3us of throwaway matmuls issued while the first
        # x chunk is still in flight, so the real matmuls start at 2.4 GHz
        dummy = fixed.tile([128, 512], bf16)
        nc.vector.memset(dummy[:], 0.0)
        warm = psum.tile([128, 2048], f32, tag="ps", name="warm")
        for _ in range(8):
            nc.tensor.matmul(warm[:, 0:512], dummy[:, 0:128], dummy[:])

        def emit_gate(t, c0, cw, q):
            """One sigmoid gate slab for pair-columns [c0, c0+cw)."""
            ps = psum.tile([128, 2048], f32, tag="ps", name=f"ps{q}")
            qs = slice(q * 64, (q + 1) * 64)
            for b0, bw in _bank_regions(cw):
                cols = slice(c0 + b0, c0 + b0 + bw)
                if t == 0:
                    # h == 0: contract over the 64 x-rows only
                    nc.tensor.matmul(ps[0:64, b0:b0 + bw],
                                     GW[0:64, qs], XA[0:64, cols])
                    nc.tensor.matmul(ps[64:128, b0:b0 + bw],
                                     GW[0:64, qs], XB[0:64, cols])
                else:
                    nc.tensor.matmul(ps[0:64, b0:b0 + bw],
                                     GW[:, qs], XA[:, cols])
                    nc.tensor.matmul(ps[64:128, b0:b0 + bw],
                                     GW[:, qs], XB[:, cols])
            g = work.tile([128, 2048], bf16, tag=f"g{q}", name=f"g{q}")
            nc.scalar.activation(g[:, 0:cw], ps[:, 0:cw], AF.Sigmoid,
                                 bias=BIAS[:, q:q + 1])
            return g

        def emit_cell(t, c0, cw, gs):
            cols = slice(c0, c0 + cw)
            sg, si, sf, so = gs[QG], gs[QI], gs[QF], gs[QO]
            tg = work.tile([128, 2048], bf16, tag="tg")
            nc.vector.tensor_scalar(tg[:, 0:cw], sg[:, 0:cw], 2.0, -1.0,
                                    ALU.mult, ALU.add)
            if t == 0:
                nc.vector.tensor_mul(C[:, cols], si[:, 0:cw], tg[:, 0:cw])
            else:
                m1 = work.tile([128, 2048], bf16, tag="m1")
                nc.vector.tensor_mul(m1[:, 0:cw], si[:, 0:cw], tg[:, 0:cw])
                m2 = work.tile([128, 2048], bf16, tag="m2")
                nc.vector.tensor_mul(m2[:, 0:cw], sf[:, 0:cw], C[:, cols])
                nc.vector.tensor_add(C[:, cols], m1[:, 0:cw], m2[:, 0:cw])
            tc_ = work.tile([128, 2048], bf16, tag="tc")
            nc.scalar.activation(tc_[:, 0:cw], C[:, cols], AF.Tanh)
            nc.vector.tensor_mul(H[:, cols], so[:, 0:cw], tc_[:, 0:cw])
            nc.vector.tensor_copy(XA[64:128, cols], H[0:64, cols])
            nc.vector.tensor_copy(XB[64:128, cols], H[64:128, cols])

        def emit_y(t, c0, cw):
            ps = psum.tile([128, 2048], f32, tag="ps")
            for b0, bw in _bank_regions(cw):
                cols = slice(c0 + b0, c0 + b0 + bw)
                nc.tensor.matmul(ps[:, b0:b0 + bw], WE[:], H[:, cols])
            yo = work.tile([128, 2048], bf16, tag="yo")
            nc.scalar.activation(yo[:, 0:cw], ps[:, 0:cw], AF.Sigmoid)
            nc.sync.dma_start(out[t, :, c0:c0 + cw], yo[:, 0:cw])

        pending = None   # (t, c0, cw) whose y is not yet emitted
        for t in range(T):
            for ci, (c0, cw) in enumerate(CHUNKS):
                gs = [emit_gate(t, c0, cw, q) for q in range(4)]
                # pad the psum rotation to 6 allocations per chunk so the
                # 2-buffer alternation stays phase-aligned across chunks
                _pad1 = psum.tile([128, 2048], f32, tag="ps", name="pad1")
                if pending is not None:
                    emit_y(*pending)
                else:
                    _pad0 = psum.tile([128, 2048], f32, tag="ps", name="pad0")
                emit_cell(t, c0, cw, gs)
                pending = (t, c0, cw)
        emit_y(*pending)

    nc.finalize()
    return nc


def _prep_shared(gate_w, gate_b, W_edge):
    """Host-side packing of the replicated weight tensors."""
    import ml_dtypes
    gw = np.asarray(gate_w, dtype=np.float32)          # (256, 128) = (4F, 2F)
    gb = np.asarray(gate_b, dtype=np.float32)          # (256,)
    we = np.asarray(W_edge, dtype=np.float32)          # (64, 64)

    gwT = gw.T                                          # (128, 256): [x;h] rows
    # reorder gate columns to (g, i, f, o); scale the g block by 2 so that
    # tanh(g) = 2*sigmoid(2g) - 1 comes out of a plain sigmoid
    gw_pack = np.concatenate(
        [2.0 * gwT[:, 128:192], gwT[:, 0:64], gwT[:, 64:128], gwT[:, 192:256]],
        axis=1,
    )

    we_pack = np.zeros((128, 128), dtype=np.float32)
    we_pack[0:64, 0:64] = we
    we_pack[64:128, 64:128] = we

    bias_pack = np.zeros((128, 4), dtype=np.float32)
    for qi, (b0, scale) in enumerate([(128, 2.0), (0, 1.0), (64, 1.0), (192, 1.0)]):
        bq = scale * gb[b0:b0 + 64]
        bias_pack[0:64, qi] = bq
        bias_pack[64:128, qi] = bq

    bf16 = ml_dtypes.bfloat16
    return gw_pack.astype(bf16), we_pack.astype(bf16), bias_pack


def kernel(inputs_edge, gate_w, gate_b, W_edge):
    from concourse.bass_utils import run_bass_kernel_spmd
    import ml_dtypes

    global _NC
    if _NC is None:
        _NC = _build()

    bf16 = ml_dtypes.bfloat16
    x_T = np.asarray(inputs_edge, dtype=np.float32).reshape(R_TOTAL, F).T
    x_T = np.ascontiguousarray(x_T).astype(bf16)        # (64, R_TOTAL)
    gw_pack, we_pack, bias_pack = _prep_shared(gate_w, gate_b, W_edge)

    in_maps = []
    for c in range(N_CORES):
        base = c * R
        in_maps.append({
            "xa": np.ascontiguousarray(x_T[:, base:base + RH]),
            "xb": np.ascontiguousarray(x_T[:, base + RH:base + R]),
            "gw": gw_pack,
            "we": we_pack,
            "bias": bias_pack,
        })

    import os
    global LAST_EXEC_NS
    trace = bool(os.environ.get("KTRACE"))
    res = run_bass_kernel_spmd(
        _NC, in_maps, core_ids=list(range(N_CORES)), trace=trace,
        trace_cores=[0] if trace else None,
    )
    if res.exec_time_ns is not None:
        LAST_EXEC_NS = res.exec_time_ns

    # per-core (T, 128, RH) bf16 -> full (T, B, N, F) f32
    full = np.empty((T, R_TOTAL, F), dtype=np.float32)
    for c in range(N_CORES):
        o = np.asarray(res.results[c]["out"]).astype(np.float32)  # (T,128,RH)
        base = c * R
        full[:, base:base + RH, :] = o[:, 0:64, :].transpose(0, 2, 1)
        full[:, base + RH:base + R, :] = o[:, 64:128, :].transpose(0, 2, 1)
    return np.ascontiguousarray(full.reshape(T, B, N, F))


# revision 28
# speedup vs baseline: 1.0004x; 1.0004x over previous
"""Trainium2 Bass kernel for nn_Decoder_LSTM: 12-step LSTM over (16, 10000, 64).

Sharding: rows = B*N = 160000 flattened, 20000 rows per core (data-parallel);
gate + edge weights replicated on all 8 cores.

Per-core layout: two 10000-row halves (A, B) packed into 128 partitions,
feature-major:
  XA/XB (128, 10000) bf16 : partitions 0:64 = x^T, 64:128 = h^T
  H     (128, 10000) bf16 : partitions 0:64 = hA,  64:128 = hB
  C     (128, 10000) bf16 : cell state, same packing as H

All four gate activations are sigmoids (tanh(g) = 2*sigmoid(2g)-1 with the
2x folded into the g-gate weights/bias), so each gate's PSUM tile (4 banks,
2048 cols) is drained by ONE ScalarE sigmoid with the per-gate bias riding
the per-partition bias operand.  Gates are time-multiplexed through a
2-buffer PSUM pool; per gate-bank two M=64 matmuls write the A-half rows to
partitions 0:64 and the B-half rows to 64:128 (no zero-padded weights, no
accumulation groups).  y = sigmoid(H @ blockdiag(We, We)) is one matmul per
bank from the shared H tile.  Output leaves the core as bf16 (T, 128, 10000)
and is unpacked/upcast on the host.
"""
import numpy as np

T, B, N, F = 12, 16, 10000, 64
R_TOTAL = B * N
N_CORES = 8
R = R_TOTAL // N_CORES   # 20000 rows per core
RH = R // 2              # 10000 per half
SC = 2048                # pair-columns per super-chunk
CHUNKS = [(i * 2000, 2000) for i in range(RH // 2000)]

_NC = None
LAST_EXEC_NS = None


def _bank_regions(cw):
    regs = []
    b0 = 0
    while b0 < cw:
        regs.append((b0, min(512, cw - b0)))
        b0 += 512
    return regs


def _build():
    from contextlib import ExitStack
    from concourse import bacc, mybir
    import concourse.tile as tile

    f32 = mybir.dt.float32
    bf16 = mybir.dt.bfloat16
    AF = mybir.ActivationFunctionType
    ALU = mybir.AluOpType

    nc = bacc.Bacc(trn_type="TRN2")
    xa_in = nc.dram_tensor("xa", [F, RH], bf16, kind="ExternalInput")
    xb_in = nc.dram_tensor("xb", [F, RH], bf16, kind="ExternalInput")
    gw_in = nc.dram_tensor("gw", [128, 256], bf16, kind="ExternalInput")
    we_in = nc.dram_tensor("we", [128, 128], bf16, kind="ExternalInput")
    bias_in = nc.dram_tensor("bias", [128, 4], f32, kind="ExternalInput")
    out = nc.dram_tensor("out", [T, 128, RH], bf16, kind="ExternalOutput")

    # gate order in the weight packing: g first so DVE can start on TG early
    # gw columns: 0:64 = g(x2), 64:128 = i, 128:192 = f, 192:256 = o
    QG, QI, QF, QO = 0, 1, 2, 3

    with tile.TileContext(nc) as tc, ExitStack() as ctx:
        fixed = ctx.enter_context(tc.tile_pool(name="fixed", bufs=1))
        state = ctx.enter_context(tc.tile_pool(name="state", bufs=1))
        work = ctx.enter_context(tc.tile_pool(name="work", bufs=2))
        psum = ctx.enter_context(tc.tile_pool(name="psum", bufs=2, space="PSUM"))

        XA = state.tile([128, RH], bf16, name="XA")
        XB = state.tile([128, RH], bf16, name="XB")
        H = state.tile([128, RH], bf16, name="H")
        C = state.tile([128, RH], bf16, name="C")
        GW = fixed.tile([128, 256], bf16)
        WE = fixed.tile([128, 128], bf16)
        BIAS = fixed.tile([128, 4], f32)

        # first-needed data first: x chunk 0, weights, then the rest
        c0_, cw_ = CHUNKS[0]
        nc.sync.dma_start(XA[0:64, c0_:c0_ + cw_], xa_in[:, c0_:c0_ + cw_])
        nc.sync.dma_start(XB[0:64, c0_:c0_ + cw_], xb_in[:, c0_:c0_ + cw_])
        nc.sync.dma_start(GW[:], gw_in[:])
        nc.sync.dma_start(BIAS[:], bias_in[:])
        nc.sync.dma_start(WE[:], we_in[:])
        for c0, cw in CHUNKS[1:]:
            nc.sync.dma_start(XA[0:64, c0:c0 + cw], xa_in[:, c0:c0 + cw])
            nc.sync.dma_start(XB[0:64, c0:c0 + cw], xb_in[:, c0:c0 + cw])

        # PE p-state warmup: ~3us of throwaway matmuls issued while the first
        # x chunk is still in flight, so the real matmuls start at 2.4 GHz
        dummy = fixed.tile([128, 512], bf16)
        nc.vector.memset(dummy[:], 0.0)
        warm = psum.tile([128, 2048], f32, tag="ps", name="warm")
        for _ in range(8):
            nc.tensor.matmul(warm[:, 0:512], dummy[:, 0:128], dummy[:])

        def emit_gate(t, c0, cw, q):
            """One sigmoid gate slab for pair-columns [c0, c0+cw)."""
            ps = psum.tile([128, 2048], f32, tag="ps", name=f"ps{q}")
            qs = slice(q * 64, (q + 1) * 64)
            for b0, bw in _bank_regions(cw):
                cols = slice(c0 + b0, c0 + b0 + bw)
                if t == 0:
                    # h == 0: contract over the 64 x-rows only
                    nc.tensor.matmul(ps[0:64, b0:b0 + bw],
                                     GW[0:64, qs], XA[0:64, cols])
                    nc.tensor.matmul(ps[64:128, b0:b0 + bw],
                                     GW[0:64, qs], XB[0:64, cols])
                else:
                    nc.tensor.matmul(ps[0:64, b0:b0 + bw],
                                     GW[:, qs], XA[:, cols])
                    nc.tensor.matmul(ps[64:128, b0:b0 + bw],
                                     GW[:, qs], XB[:, cols])
            g = work.tile([128, 2048], bf16, tag=f"g{q}", name=f"g{q}")
            nc.scalar.activation(g[:, 0:cw], ps[:, 0:cw], AF.Sigmoid,
                                 bias=BIAS[:, q:q + 1])
            return g

        def emit_cell(t, c0, cw, gs):
            cols = slice(c0, c0 + cw)
            sg, si, sf, so = gs[QG], gs[QI], gs[QF], gs[QO]
            tg = work.tile([128, 2048], bf16, tag="tg")
            nc.vector.tensor_scalar(tg[:, 0:cw], sg[:, 0:cw], 2.0, -1.0,
                                    ALU.mult, ALU.add)
            if t == 0:
                nc.vector.tensor_mul(C[:, cols], si[:, 0:cw], tg[:, 0:cw])
            else:
                m1 = work.tile([128, 2048], bf16, tag="m1")
                nc.vector.tensor_mul(m1[:, 0:cw], si[:, 0:cw], tg[:, 0:cw])
                m2 = work.tile([128, 2048], bf16, tag="m2")
                nc.vector.tensor_mul(m2[:, 0:cw], sf[:, 0:cw], C[:, cols])
                nc.vector.tensor_add(C[:, cols], m1[:, 0:cw], m2[:, 0:cw])
            tc_ = work.tile([128, 2048], bf16, tag="tc")
            nc.scalar.activation(tc_[:, 0:cw], C[:, cols], AF.Tanh)
            nc.vector.tensor_mul(H[:, cols], so[:, 0:cw], tc_[:, 0:cw])
            nc.vector.tensor_copy(XA[64:128, cols], H[0:64, cols])
            nc.vector.tensor_copy(XB[64:128, cols], H[64:128, cols])

        def emit_y(t, c0, cw):
            ps = psum.tile([128, 2048], f32, tag="ps")
            for b0, bw in _bank_regions(cw):
                cols = slice(c0 + b0, c0 + b0 + bw)
                nc.tensor.matmul(ps[:, b0:b0 + bw], WE[:], H[:, cols])
            yo = work.tile([128, 2048], bf16, tag="yo")
            nc.scalar.activation(yo[:, 0:cw], ps[:, 0:cw], AF.Sigmoid)
            nc.sync.dma_start(out[t, :, c0:c0 + cw], yo[:, 0:cw])

        def emit_y_final(t, c0, cw):
            # last chunk of the run: split sigma+DMA in two so the output
            # starts draining while the second half is still activating
            ps = psum.tile([128, 2048], f32, tag="ps")
            for b0, bw in _bank_regions(cw):
                cols = slice(c0 + b0, c0 + b0 + bw)
                nc.tensor.matmul(ps[:, b0:b0 + bw], WE[:], H[:, cols])
            yo = work.tile([128, 2048], bf16, tag="yo")
            half = 1024
            for p0, pw in ((0, half), (half, cw - half)):
                nc.scalar.activation(yo[:, p0:p0 + pw], ps[:, p0:p0 + pw],
                                     AF.Sigmoid)
                nc.sync.dma_start(out[t, :, c0 + p0:c0 + p0 + pw],
                                  yo[:, p0:p0 + pw])

        pending = None   # (t, c0, cw) whose y is not yet emitted
        for t in range(T):
            for ci, (c0, cw) in enumerate(CHUNKS):
                gs = [emit_gate(t, c0, cw, q) for q in range(4)]
                # pad the psum rotation to 6 allocations per chunk so the
                # 2-buffer alternation stays phase-aligned across chunks
                _pad1 = psum.tile([128, 2048], f32, tag="ps", name="pad1")
                if pending is not None:
                    emit_y(*pending)
                else:
                    _pad0 = psum.tile([128, 2048], f32, tag="ps", name="pad0")
                emit_cell(t, c0, cw, gs)
                pending = (t, c0, cw)
        emit_y_final(*pending)

    nc.finalize()
    return nc


def _prep_shared(gate_w, gate_b, W_edge):
    """Host-side packing of the replicated weight tensors."""
    import ml_dtypes
    gw = np.asarray(gate_w, dtype=np.float32)          # (256, 128) = (4F, 2F)
    gb = np.asarray(gate_b, dtype=np.float32)          # (256,)
    we = np.asarray(W_edge, dtype=np.float32)          # (64, 64)

    gwT = gw.T                                          # (128, 256): [x;h] rows
    # reorder gate columns to (g, i, f, o); scale the g block by 2 so that
    # tanh(g) = 2*sigmoid(2g) - 1 comes out of a plain sigmoid
    gw_pack = np.concatenate(
        [2.0 * gwT[:, 128:192], gwT[:, 0:64], gwT[:, 64:128], gwT[:, 192:256]],
        axis=1,
    )

    we_pack = np.zeros((128, 128), dtype=np.float32)
    we_pack[0:64, 0:64] = we
    we_pack[64:128, 64:128] = we

    bias_pack = np.zeros((128, 4), dtype=np.float32)
    for qi, (b0, scale) in enumerate([(128, 2.0), (0, 1.0), (64, 1.0), (192, 1.0)]):
        bq = scale * gb[b0:b0 + 64]
        bias_pack[0:64, qi] = bq
        bias_pack[64:128, qi] = bq

    bf16 = ml_dtypes.bfloat16
    return gw_pack.astype(bf16), we_pack.astype(bf16), bias_pack


def kernel(inputs_edge, gate_w, gate_b, W_edge):
    from concourse.bass_utils import run_bass_kernel_spmd
    import ml_dtypes

    global _NC
    if _NC is None:
        _NC = _build()

    bf16 = ml_dtypes.bfloat16
    x_T = np.asarray(inputs_edge, dtype=np.float32).reshape(R_TOTAL, F).T
    x_T = np.ascontiguousarray(x_T).astype(bf16)        # (64, R_TOTAL)
    gw_pack, we_pack, bias_pack = _prep_shared(gate_w, gate_b, W_edge)

    in_maps = []
    for c in range(N_CORES):
        base = c * R
        in_maps.append({
            "xa": np.ascontiguousarray(x_T[:, base:base + RH]),
            "xb": np.ascontiguousarray(x_T[:, base + RH:base + R]),
            "gw": gw_pack,
            "we": we_pack,
            "bias": bias_pack,
        })

    import os
    global LAST_EXEC_NS
    trace = bool(os.environ.get("KTRACE"))
    res = run_bass_kernel_spmd(
        _NC, in_maps, core_ids=list(range(N_CORES)), trace=trace,
        trace_cores=[0] if trace else None,
    )
    if res.exec_time_ns is not None:
        LAST_EXEC_NS = res.exec_time_ns

    # per-core (T, 128, RH) bf16 -> full (T, B, N, F) f32
    full = np.empty((T, R_TOTAL, F), dtype=np.float32)
    for c in range(N_CORES):
        o = np.asarray(res.results[c]["out"]).astype(np.float32)  # (T,128,RH)
        base = c * R
        full[:, base:base + RH, :] = o[:, 0:64, :].transpose(0, 2, 1)
        full[:, base + RH:base + R, :] = o[:, 64:128, :].transpose(0, 2, 1)
    return np.ascontiguousarray(full.reshape(T, B, N, F))
